# revision 1
# baseline (speedup 1.0000x reference)
"""Trainium2 Bass kernel for ArgKeyFactIndex batched segment-index lookup.

Problem: B queries (pred, a0, a1); each selects one of three segment-index
tables ((pred,a0), (pred,a1), pred-only), looks up (start, len) for its key,
and gathers max_results=64 consecutive fact indices from that table's order
array (clipped at the end), plus a validity mask.

Strategy: data-parallel over the query batch across 8 NeuronCores; the
read-only tables are replicated per core. On each core:
  1. vector engine computes the selected table key / order-array base /
     gate per query (int32 ops, all values < 2^24 so exact in any ALU path)
  2. indirect-DMA gathers fetch the (start, len) pair per query from an
     interleaved starts/lens table (the HW indirect DMA consumes one
     offset per partition, so one instruction per 128 queries)
  3. indirect-DMA gathers fetch the 64 consecutive int32 fact indices per
     query from a concatenated order array (each segment padded with 64
     copies of its last element, which reproduces the reference's index
     clipping exactly)
  4. valid mask = (iota64 < effective_count) via DVE compares that overlap
     the gather stream; work is chunked over query columns so gathers,
     vector math and store DMAs pipeline across chunks
Results are re-assembled host-side. The kernel is Q7 descriptor-generation
bound (~1.1us per 128-descriptor indirect DMA).
"""

import numpy as np

import concourse.bass as bass
import concourse.bacc as bacc
import concourse.tile as tile
import concourse.mybir as mybir
from concourse.bass_utils import run_bass_kernel_spmd

CNO = 10000      # constant_no
PAD = 10001      # padding / 'variable' marker
KS = 10003       # key pack base
K = 64           # max_results
NCORES = 8
P = 128

# test harness hooks (kernel() itself never sets these)
TRACE = False
LAST_RESULTS = None

_cache = {}


def _pick_chunk(C):
    for cs in range(min(C, 32), 0, -1):
        if C % cs == 0:
            return cs
    return C


def _build(T0, T1, Tp, F, C):
    """Build + compile the per-core Bass program. All 8 cores run the same
    NEFF on different query shards."""
    i32 = mybir.dt.int32
    u8 = mybir.dt.uint8
    TT = T0 + T1 + Tp
    OL = 3 * (F + K)
    cs = _pick_chunk(C)          # queries-per-partition per chunk
    nchunks = C // cs

    nc = bacc.Bacc("TRN2", target_bir_lowering=False, debug=False,
                   num_devices=NCORES)

    qp_d = nc.dram_tensor("qp", [P, C], i32, kind="ExternalInput")
    qa0_d = nc.dram_tensor("qa0", [P, C], i32, kind="ExternalInput")
    qa1_d = nc.dram_tensor("qa1", [P, C], i32, kind="ExternalInput")
    sl_d = nc.dram_tensor("sl_cat", [TT, 2], i32, kind="ExternalInput")
    ord_d = nc.dram_tensor("order_cat", [OL, 1], i32, kind="ExternalInput")
    fact_d = nc.dram_tensor("fact", [P, C * K], i32, kind="ExternalOutput")
    valid_d = nc.dram_tensor("valid", [P, C * K], u8, kind="ExternalOutput")

    with tile.TileContext(nc) as tc:
        with (
            tc.tile_pool(name="keys", bufs=1) as keys_pool,
            tc.tile_pool(name="slg", bufs=3) as slg_pool,
            tc.tile_pool(name="mid", bufs=3) as mid_pool,
            tc.tile_pool(name="big", bufs=3) as big_pool,
        ):
            qp = keys_pool.tile([P, C], i32)
            qa0 = keys_pool.tile([P, C], i32)
            qa1 = keys_pool.tile([P, C], i32)
            nc.sync.dma_start(qp[:], qp_d.ap())
            nc.sync.dma_start(qa0[:], qa0_d.ap())
            nc.sync.dma_start(qa1[:], qa1_d.ap())

            A = mybir.AluOpType

            def key_math(csl):
                """Per-chunk key computation on [P, cs] tiles, so chunk 0's
                gathers become eligible after 1/nchunks of the prologue."""
                isc0 = mid_pool.tile([P, cs], i32, tag="isc0")
                bv = mid_pool.tile([P, cs], i32, tag="bv")
                gate = mid_pool.tile([P, cs], i32, tag="gatec")
                tmp = mid_pool.tile([P, cs], i32, tag="tmpc")
                tmp2 = mid_pool.tile([P, cs], i32, tag="tmp2c")
                gkey = mid_pool.tile([P, cs], i32, tag="gkeyc")
                obase = mid_pool.tile([P, cs], i32, tag="obasec")
                kb = mid_pool.tile([P, cs], i32, tag="kbc")
                key0 = mid_pool.tile([P, cs], i32, tag="key0c")
                key1 = mid_pool.tile([P, cs], i32, tag="key1c")
                # is_c0 = a0 <= CNO  (PAD > CNO so the reference's extra
                # a0 != PAD term is redundant for any int input)
                nc.vector.tensor_scalar(isc0[:], qa0[:, csl], CNO, None,
                                        op0=A.is_le)
                # both_var = (~is_c0) & (~is_c1) & (pred != PAD)
                nc.vector.tensor_scalar(tmp[:], qa0[:, csl], CNO, None,
                                        op0=A.is_gt)
                nc.vector.tensor_scalar(tmp2[:], qa1[:, csl], CNO, None,
                                        op0=A.is_gt)
                nc.vector.tensor_tensor(bv[:], tmp[:], tmp2[:], op=A.mult)
                nc.vector.tensor_scalar(tmp[:], qp[:, csl], PAD, None,
                                        op0=A.not_equal)
                nc.vector.tensor_tensor(bv[:], bv[:], tmp[:], op=A.mult)
                # gate = is_c0 | is_c1 | both_var
                nc.vector.tensor_scalar(tmp[:], qa1[:, csl], CNO, None,
                                        op0=A.is_le)
                nc.vector.tensor_tensor(gate[:], isc0[:], tmp[:], op=A.max)
                nc.vector.tensor_tensor(gate[:], gate[:], bv[:], op=A.max)
                # keys: key0 = qp*KS + qa0 ; key1 = qp*KS + qa1 ; keyp = qp
                nc.vector.tensor_scalar(kb[:], qp[:, csl], KS, None,
                                        op0=A.mult)
                nc.vector.tensor_tensor(key0[:], kb[:], qa0[:, csl], op=A.add)
                nc.vector.tensor_tensor(key1[:], kb[:], qa1[:, csl], op=A.add)
                # clip to each table's range: clip(key, 0, T-1)
                nc.vector.tensor_scalar(key0[:], key0[:], 0, T0 - 1,
                                        op0=A.max, op1=A.min)
                nc.vector.tensor_scalar(key1[:], key1[:], 0, T1 - 1,
                                        op0=A.max, op1=A.min)
                # gkey: concatenated-table key.  default = key1 + T0,
                # overridden by isc0 -> key0, by bv -> clip(qp) + T0 + T1
                nc.vector.tensor_scalar(gkey[:], key1[:], T0, None,
                                        op0=A.add)
                nc.vector.copy_predicated(gkey[:], isc0[:], key0[:])
                nc.vector.tensor_scalar(tmp[:], qp[:, csl], 0, Tp - 1,
                                        op0=A.max, op1=A.min)
                nc.vector.tensor_scalar(tmp[:], tmp[:], T0 + T1, None,
                                        op0=A.add)
                nc.vector.copy_predicated(gkey[:], bv[:], tmp[:])
                # order-array base: tsel = 1 - isc0 + bv in {0,1,2};
                # obase = tsel * (F+K)
                nc.vector.tensor_scalar(tmp[:], isc0[:], -1, 1, op0=A.mult,
                                        op1=A.add)
                nc.vector.tensor_tensor(tmp[:], tmp[:], bv[:], op=A.add)
                nc.vector.tensor_scalar(obase[:], tmp[:], F + K, None,
                                        op0=A.mult)
                return gkey, obase, gate

            # iota64 block pattern (built once, broadcast per chunk)
            iota64 = keys_pool.tile([P, K], i32)
            nc.gpsimd.iota(iota64[:], pattern=[[1, K]], base=0,
                           channel_multiplier=0)

            for ch in range(nchunks):
                csl = slice(ch * cs, (ch + 1) * cs)
                gkey, obase, gate = key_math(csl)  # per-chunk [P, cs] tiles
                # (start, len) pair gather for this chunk's queries.
                # HW indirect DMA consumes ONE offset per partition per
                # instruction, so issue one per column.
                slt = slg_pool.tile([P, cs * 2], i32, tag="slt")
                for c in range(cs):
                    nc.gpsimd.indirect_dma_start(
                        out=slt[:, 2 * c:2 * c + 2],
                        out_offset=None,
                        in_=sl_d.ap(),
                        in_offset=bass.IndirectOffsetOnAxis(
                            ap=gkey[:, c:c + 1], axis=0),
                    )
                leftg = mid_pool.tile([P, cs], i32, tag="leftg")
                effcnt = mid_pool.tile([P, cs], i32, tag="effcnt")
                nc.vector.tensor_tensor(leftg[:], slt[:, 0::2],
                                        obase[:], op=A.add)
                nc.vector.tensor_scalar(effcnt[:], slt[:, 1::2], K, None,
                                        op0=A.min)
                nc.vector.tensor_tensor(effcnt[:], effcnt[:], gate[:],
                                        op=A.mult)

                # the big gather: 64 consecutive fact indices per query
                fact = big_pool.tile([P, cs * K], i32, tag="fact")
                for c in range(cs):
                    nc.gpsimd.indirect_dma_start(
                        out=fact[:, c * K:(c + 1) * K],
                        out_offset=None,
                        in_=ord_d.ap(),
                        in_offset=bass.IndirectOffsetOnAxis(
                            ap=leftg[:, c:c + 1], axis=0),
                    )

                valid = big_pool.tile([P, cs * K], u8, tag="valid")
                nc.vector.tensor_tensor(
                    out=valid[:].rearrange("p (c e) -> p c e", e=K),
                    in0=iota64[:].rearrange("p (o e) -> p o e", o=1)
                        .to_broadcast([P, cs, K]),
                    in1=effcnt[:].to_broadcast([P, cs, K]),
                    op=A.is_lt,  # valid = iota < cnt
                )

                nc.sync.dma_start(fact_d.ap()[:, ch * cs * K:(ch + 1) * cs * K],
                                  fact[:])
                nc.sync.dma_start(valid_d.ap()[:, ch * cs * K:(ch + 1) * cs * K],
                                  valid[:])

    nc.compile()
    return nc


def kernel(query_atoms, a0_order, a0_starts, a0_lens,
           a1_order, a1_starts, a1_lens,
           p_order, p_starts, p_lens, max_results=64):
    global LAST_RESULTS
    qa = np.asarray(query_atoms, dtype=np.int32)
    o0 = np.asarray(a0_order, dtype=np.int32).ravel()
    s0 = np.asarray(a0_starts, dtype=np.int32).ravel()
    l0 = np.asarray(a0_lens, dtype=np.int32).ravel()
    o1 = np.asarray(a1_order, dtype=np.int32).ravel()
    s1 = np.asarray(a1_starts, dtype=np.int32).ravel()
    l1 = np.asarray(a1_lens, dtype=np.int32).ravel()
    op_ = np.asarray(p_order, dtype=np.int32).ravel()
    sp = np.asarray(p_starts, dtype=np.int32).ravel()
    lp = np.asarray(p_lens, dtype=np.int32).ravel()
    assert int(np.asarray(max_results)) == K

    B = qa.shape[0]
    F = o0.size
    T0, T1, Tp = s0.size, s1.size, sp.size
    n_per = -(-B // NCORES)          # queries per core (pre-padding)
    C = -(-n_per // P)               # columns per partition
    bpad = P * C

    key = (T0, T1, Tp, F, C)
    if key not in _cache:
        _cache[key] = _build(*key)
    nc = _cache[key]

    # interleaved (start, len) pairs for the three tables, concatenated
    sl_cat = np.empty((T0 + T1 + Tp, 2), np.int32)
    sl_cat[:T0, 0], sl_cat[:T0, 1] = s0, l0
    sl_cat[T0:T0 + T1, 0], sl_cat[T0:T0 + T1, 1] = s1, l1
    sl_cat[T0 + T1:, 0], sl_cat[T0 + T1:, 1] = sp, lp

    # concatenated order arrays, each padded with K copies of its last
    # element so a contiguous 64-read reproduces clip(left+j, 0, F-1)
    order_cat = np.empty((3 * (F + K), 1), np.int32)
    for i, o in enumerate((o0, o1, op_)):
        base = i * (F + K)
        order_cat[base:base + F, 0] = o
        order_cat[base + F:base + F + K, 0] = o[-1]

    in_maps = []
    for i in range(NCORES):
        lo, hi = i * n_per, min((i + 1) * n_per, B)
        shard = np.empty((bpad, 3), np.int32)
        shard[:hi - lo] = qa[lo:hi]
        shard[hi - lo:] = (0, 1, PAD)          # harmless pad queries
        in_maps.append({
            "qp": np.ascontiguousarray(shard[:, 0].reshape(P, C)),
            "qa0": np.ascontiguousarray(shard[:, 1].reshape(P, C)),
            "qa1": np.ascontiguousarray(shard[:, 2].reshape(P, C)),
            "sl_cat": sl_cat,
            "order_cat": order_cat,
        })

    res = run_bass_kernel_spmd(nc, in_maps, core_ids=list(range(NCORES)),
                               trace=TRACE)
    LAST_RESULTS = res

    fact_full = np.empty((B, K), np.int32)
    valid_full = np.empty((B, K), bool)
    for i in range(NCORES):
        lo, hi = i * n_per, min((i + 1) * n_per, B)
        r = res.results[i]
        fact_full[lo:hi] = r["fact"].reshape(bpad, K)[:hi - lo]
        valid_full[lo:hi] = r["valid"].reshape(bpad, K)[:hi - lo].astype(bool)
    return fact_full, valid_full

